# revision 41
# baseline (speedup 1.0000x reference)
"""Cross-WindowAttention Trainium2 kernel.

Full inputs -> shard batch dim over 8 NeuronCores -> bass/Tile kernel per core
-> gather. Host-side numpy does layout prep (transposes to feature-major,
bf16 conversion, combined rpb+mask bias table); the Bass kernel does all
matmul/softmax compute.

Per-core pipeline (shard = 256 windows of 64 tokens, 16384 rows):
 - qkv projections on PE in bf16, contraction over concat(x,y) for k/v.
   q,k produced feature-major [feat, rows]; v row-major per window-pair
   [128, 512], duplicated (pair-swapped) to the other column block via two
   SBUF->SBUF DMAs so PV can run 4-way tiled.
 - attention per (head-quad, 8-window chunk) in one [128, 1024] PSUM tile,
   with QK^T emitted TRANSPOSED (k stationary) so attn lands [keys, queries]
   and no PE transpose is ever needed.  The four heads of a quad rotate
   through all four 32-row PE array groups -> 4-way concurrent sub-tile
   matmuls for both QK and PV.
 - softmax: combined rpb+mask bias applied multiplicatively on DVE
   (exp(s*qk+b) = exp(s*qk)*exp(b), exp(b) precomputed); denominators via
   two masked-ones matmuls whose stationary masks REPLICATE each key-half
   sum across a 32-partition block, so den lands in PSUM already broadcast
   to the PV-output layout; one fast reciprocal on DVE then feeds the
   normalization multiply directly -- no separate broadcast matmul, no
   bf16 cast.
 - output projection with attention-output tiles stationary -> row-major
   result, biases via DVE add, contiguous DMA out.

Round schedule (software-pipelined two chunks deep): round c emits, in
interleaved slots against the k/v projection groups of chunk c: the QK
matmuls of chunk c-1 (slots 0-3), its denominator matmuls (slots 4-7, by
which time exp+bias-mult have long finished), its broadcast+PV+normalize
(slots 8-11, recip long finished), then the output projections of chunk
c-2 against the cheap q-projection groups.  Every PE instruction's
cross-engine dependencies are thus ~10us stale when the PE reaches it --
no head-of-line stalls in the PE queue.
"""
import numpy as np
import ml_dtypes

import concourse.bacc as bacc
import concourse.mybir as mybir
from concourse.tile import TileContext
from concourse.bass_utils import run_bass_kernel_spmd

F32 = mybir.dt.float32
BF16 = mybir.dt.bfloat16
BF = ml_dtypes.bfloat16

N_CORES = 8
B_FULL = 2048
N = 64                      # window size (tokens per window)
C = 512                     # channels
H = 16                      # heads
HD = 32                     # head dim
CX = 512                    # x feature dim
CY = 1000                   # y feature dim
CYP = 1024                  # y feature dim padded to multiple of 128
SCALE = HD ** -0.5

B_SHARD = B_FULL // N_CORES             # 256 windows per core
WIN_PER_CHUNK = 8
ROWS_PER_CHUNK = WIN_PER_CHUNK * N      # 512
N_CHUNKS = B_SHARD // WIN_PER_CHUNK     # 32

KT_X = CX // 128            # 4 contraction tiles from x
KT_Y = CYP // 128           # 8 contraction tiles from y (padded)
FT_Q = C // 128             # 4 feature tiles per projection output


def build_nc(n_chunks=N_CHUNKS):
    rows = n_chunks * ROWS_PER_CHUNK
    nc = bacc.Bacc("TRN2", target_bir_lowering=False)

    xt = nc.dram_tensor("xt", [CX, rows], BF16, kind="ExternalInput")
    yt = nc.dram_tensor("yt", [CYP, rows], BF16, kind="ExternalInput")
    w1 = nc.dram_tensor("w1", [CX, 3 * C], BF16, kind="ExternalInput")
    w2 = nc.dram_tensor("w2", [CYP, 3 * C], BF16, kind="ExternalInput")
    wp = nc.dram_tensor("wp", [4, 128, C], BF16, kind="ExternalInput")  # quad-permuted rows
    cb = nc.dram_tensor("cb", [8, 8, 128, 512], BF16, kind="ExternalInput")
    bq = nc.dram_tensor("bq", [128, FT_Q], F32, kind="ExternalInput")
    bp = nc.dram_tensor("bp", [128, C], F32, kind="ExternalInput")
    onesr = nc.dram_tensor("onesr", [128, 2, 128], BF16, kind="ExternalInput")
    out = nc.dram_tensor("out", [rows, C], F32, kind="ExternalOutput")

    with TileContext(nc) as tc:
        with tc.tile_pool(name="const", bufs=1) as constp, \
             tc.tile_pool(name="wpool", bufs=1) as wpool, \
             tc.tile_pool(name="stream", bufs=2) as stream, \
             tc.tile_pool(name="acts", bufs=2) as acts, \
             tc.tile_pool(name="small", bufs=3) as small, \
             tc.tile_pool(name="pbig", bufs=3, space="PSUM") as pbig, \
             tc.tile_pool(name="pattn", bufs=1, space="PSUM") as pattn, \
             tc.tile_pool(name="pden", bufs=1, space="PSUM") as pden, \
             tc.tile_pool(name="pot", bufs=2, space="PSUM") as pot:

            # ---- resident constants / weights
            w1_sb = wpool.tile([128, KT_X, 3 * C], BF16)
            nc.sync.dma_start(out=w1_sb, in_=w1.rearrange("(a p) f -> p a f", p=128))
            w2_sb = wpool.tile([128, KT_Y, 3 * C], BF16)
            nc.sync.dma_start(out=w2_sb, in_=w2.rearrange("(a p) f -> p a f", p=128))
            wp_sb = wpool.tile([128, 4, C], BF16)
            nc.sync.dma_start(out=wp_sb, in_=wp.rearrange("a p f -> p a f"))
            bq_sb = constp.tile([128, FT_Q], F32)
            nc.sync.dma_start(out=bq_sb, in_=bq[:, :])
            bp_sb = constp.tile([128, C], F32)
            nc.sync.dma_start(out=bp_sb, in_=bp[:, :])
            onesr_sb = constp.tile([128, 2, 128], BF16)
            nc.sync.dma_start(out=onesr_sb, in_=onesr.rearrange("p a f -> p (a f)"))

            xt_r = xt.rearrange("(a p) r -> p a r", p=128)
            yt_r = yt.rearrange("(a p) r -> p a r", p=128)

            st = {}  # per-chunk live tiles
            pending_out = []  # deferred output DMA triggers

            def emit_dma(c):
                r0 = c * ROWS_PER_CHUNK
                s = {}
                s["xt"] = stream.tile([128, KT_X, ROWS_PER_CHUNK], BF16, tag="xt", name="xt")
                nc.sync.dma_start(out=s["xt"], in_=xt_r[:, :, r0:r0 + ROWS_PER_CHUNK])
                s["yt"] = stream.tile([128, KT_Y, ROWS_PER_CHUNK], BF16, tag="yt", name="yt")
                nc.sync.dma_start(out=s["yt"], in_=yt_r[:, :, r0:r0 + ROWS_PER_CHUNK])
                s["cb"] = stream.tile([128, 8, 512], BF16, tag="cb", name="cbt")
                nc.sync.dma_start(out=s["cb"],
                                  in_=cb[c % 8].rearrange("hp p f -> p hp f"))
                s["q"] = acts.tile([128, FT_Q, ROWS_PER_CHUNK], BF16, tag="q", name="qsb")
                s["k"] = acts.tile([128, FT_Q, ROWS_PER_CHUNK], BF16, tag="k", name="ksb")
                s["v"] = acts.tile([128, WIN_PER_CHUNK, C], BF16, tag="v", name="vsb")
                s["ot"] = acts.tile([128, 4, ROWS_PER_CHUNK], BF16, tag="ot", name="otsb")
                s["expa"] = [None] * 4
                s["rden"] = [None] * 4
                st[c] = s

            def emit_qkv_group(c, g):
                s = st[c]
                if g < FT_Q:                      # q projection, feature tile g
                    ft = g
                    bank = pbig.tile([128, ROWS_PER_CHUNK], F32, tag="pq")
                    for kt in range(KT_X):
                        nc.tensor.matmul(
                            bank[:, :],
                            w1_sb[:, kt, 128 * ft:128 * (ft + 1)],
                            s["xt"][:, kt, :],
                            start=(kt == 0), stop=(kt == KT_X - 1))
                    nc.scalar.activation(
                        s["q"][:, ft, :], bank[:, :],
                        mybir.ActivationFunctionType.Identity,
                        bias=bq_sb[:, ft:ft + 1])
                elif g < 2 * FT_Q:                # k projection, feature tile g-4
                    ft = g - FT_Q
                    bank = pbig.tile([128, ROWS_PER_CHUNK], F32, tag="pq")
                    for kt in range(KT_X):
                        nc.tensor.matmul(
                            bank[:, :],
                            w1_sb[:, kt, C + 128 * ft:C + 128 * (ft + 1)],
                            s["xt"][:, kt, :],
                            start=(kt == 0), stop=False)
                    for kt in range(KT_Y):
                        nc.tensor.matmul(
                            bank[:, :],
                            w2_sb[:, kt, C + 128 * ft:C + 128 * (ft + 1)],
                            s["yt"][:, kt, :],
                            start=False, stop=(kt == KT_Y - 1))
                    nc.scalar.copy(s["k"][:, ft, :], bank[:, :])
                else:                             # v projection, window pair g-8
                    rt = g - 2 * FT_Q
                    bank = pbig.tile([128, C], F32, tag="pq")
                    for kt in range(KT_X):
                        nc.tensor.matmul(
                            bank[:, :],
                            s["xt"][:, kt, 128 * rt:128 * (rt + 1)],
                            w1_sb[:, kt, 2 * C:3 * C],
                            start=(kt == 0), stop=False)
                    for kt in range(KT_Y):
                        nc.tensor.matmul(
                            bank[:, :],
                            s["yt"][:, kt, 128 * rt:128 * (rt + 1)],
                            w2_sb[:, kt, 2 * C:3 * C],
                            start=False, stop=(kt == KT_Y - 1))
                    # block 2rt holds (win 2rt | win 2rt+1) on the two
                    # partition halves; block 2rt+1 the pair-swapped copy
                    # (made by two SBUF->SBUF DMAs) so PV finds each window
                    # on BOTH halves -> 4-way tiled PV
                    nc.scalar.copy(s["v"][:, 2 * rt, :], bank[:, :])
                    nc.sync.dma_start(out=s["v"][0:64, 2 * rt + 1, :],
                                      in_=s["v"][64:128, 2 * rt, :])
                    nc.sync.dma_start(out=s["v"][64:128, 2 * rt + 1, :],
                                      in_=s["v"][0:64, 2 * rt, :])

            def emit_attn_qk(c, qd):
                """QK^T + exp + bias-mult for heads 4qd..4qd+3."""
                s = st[c]
                # one 2-bank PSUM tile: cols [0:512] = heads (4qd, 4qd+1),
                # cols [512:1024] = heads (4qd+2, 4qd+3); [keys m, queries n]
                bank = pattn.tile([128, 1024], F32, tag="pattn")
                # QK^T transposed: k stationary -> out[keys m, queries n].
                # heads rotate through all 4 row groups -> 4-way concurrency.
                for sw in range(WIN_PER_CHUNK):
                    for hh2 in range(4):
                        pq = 32 * hh2
                        hh = hh2 % 2
                        o0 = 512 * (hh2 // 2) + 64 * sw
                        nc.tensor.matmul(
                            bank[64 * hh:64 * (hh + 1), o0:o0 + 64],
                            s["k"][pq:pq + 32, qd, 64 * sw:64 * (sw + 1)],
                            s["q"][pq:pq + 32, qd, 64 * sw:64 * (sw + 1)],
                            start=True, stop=True, skip_group_check=True,
                            tile_position=(pq, 64 * hh))
                # exp straight off the QK psum (frees the bank early);
                # the rpb+mask bias is applied multiplicatively afterwards:
                # exp(s*qk + b) = exp(s*qk) * exp(b), with exp(b) precomputed
                eraw = small.tile([128, 1024], BF16, tag="eraw")
                nc.scalar.activation(eraw, bank[:, :],
                                     mybir.ActivationFunctionType.Exp,
                                     scale=SCALE)
                expa = small.tile([128, 1024], BF16, tag="expa", bufs=4)
                nc.vector.tensor_tensor(
                    out=expa[:, :], in0=eraw[:, :],
                    in1=s["cb"][:, 2 * qd:2 * qd + 2, :].rearrange("p j f -> p (j f)"),
                    op=mybir.AluOpType.mult)
                s["expa"][qd] = expa

            def emit_attn_den(c, qd):
                """Softmax denominators, pre-broadcast, + reciprocal.

                The two masked-ones matmuls write den_rep[m, n] =
                den(head m//32, query n) -- the stationary mask columns
                replicate each key-half sum across a 32-partition block, so
                no separate broadcast matmul is ever needed.  Rows 0-63 come
                from expa's first column half (head pair 0), rows 64-127
                from the second (one accumulation group, disjoint rows).
                """
                s = st[c]
                expa = s["expa"][qd]
                denr = pden.tile([128, 512], F32, tag="pden")
                nc.tensor.matmul(denr[:, :], onesr_sb[:, 0, :], expa[:, 0:512],
                                 start=True, stop=False)
                nc.tensor.matmul(denr[:, :], onesr_sb[:, 1, :], expa[:, 512:1024],
                                 start=False, stop=True)
                rden = small.tile([128, 512], F32, tag="rden", bufs=4)
                nc.vector.reciprocal_approx_fast(rden[:, :], denr[:, :])
                s["rden"][qd] = rden

            def emit_attn_pv(c, qd):
                """PV + normalize for heads 4qd..4qd+3.

                PV on unnormalized weights: v stationary, exp moving."""
                s = st[c]
                expa = s["expa"][qd]
                # out^T[hd, n] per (head, window) -> one [128, 512] bank/quad.
                obank = pot.tile([128, 512], F32, tag="pot")
                for sw in range(WIN_PER_CHUNK):
                    for hh2 in range(4):
                        h = 4 * qd + hh2
                        half = 64 * (hh2 % 2)
                        e0 = 512 * (hh2 // 2) + 64 * sw
                        nc.tensor.matmul(
                            obank[32 * hh2:32 * (hh2 + 1), 64 * sw:64 * (sw + 1)],
                            s["v"][half:half + 64, sw ^ (hh2 % 2), HD * h:HD * (h + 1)],
                            expa[half:half + 64, e0:e0 + 64],
                            start=True, stop=True, skip_group_check=True,
                            tile_position=(half, 32 * hh2))
                # stage the raw PV output to SBUF, then normalize against the
                # pre-broadcast reciprocal (both SBUF operands)
                ob_sb = small.tile([128, 512], F32, tag="obsb")
                nc.scalar.copy(ob_sb, obank[:, :])
                nc.vector.tensor_tensor(out=s["ot"][:, qd, :],
                                        in0=s["rden"][qd][:, :],
                                        in1=ob_sb[:, :],
                                        op=mybir.AluOpType.mult)

            def emit_proj_group(c, rt):
                s = st[c]
                r0 = c * ROWS_PER_CHUNK
                bank = pbig.tile([128, C], F32, tag="pq")
                for quad in range(4):
                    nc.tensor.matmul(
                        bank[:, :],
                        s["ot"][:, quad, 128 * rt:128 * (rt + 1)],
                        wp_sb[:, quad, :],
                        start=(quad == 0), stop=(quad == 3))
                out_f32 = small.tile([128, C], F32, tag="outf", bufs=6)
                nc.vector.tensor_tensor(out=out_f32[:, :], in0=bank[:, :],
                                        in1=bp_sb[:, :], op=mybir.AluOpType.add)
                # defer the DMA trigger to the end of the round: by then the
                # bias add has long finished, so the trigger's semaphore wait
                # is ~0 and it cannot head-of-line block the next round's
                # input DMA triggers on the sync queue
                pending_out.append((out_f32, r0 + 128 * rt))

            # software pipeline, two chunks deep: round c runs the qkv
            # projections of chunk c, the attention of chunk c-1 (QK early,
            # PV half a round later), and the output projection of chunk c-2.
            for c in range(n_chunks + 2):
                if c < n_chunks:
                    emit_dma(c)
                # output DMA triggers of the previous round's proj groups:
                # AFTER this round's input triggers (so they never delay the
                # input prefetch), and by now their producers are done, so
                # the sync queue never stalls on them
                for out_f32, row in pending_out:
                    nc.sync.dma_start(out=out[row:row + 128, :], in_=out_f32[:, :])
                pending_out.clear()
                # slot plan: the cheap q-projection groups pair with the
                # early qk/den slots (whose cross-engine chains have slack);
                # every pv slot is followed by a LONG k/v group so the
                # ob-copy + normalize chain of quad q finishes before quad
                # q+1's broadcast matmul needs the denb PSUM bank back
                smalls = []
                if 1 <= c <= n_chunks:
                    a = c - 1
                    smalls = [[("qk", a, 0)], [("qk", a, 1)], [("den", a, 0)],
                              [("qk", a, 2)], [("den", a, 1)], [("qk", a, 3)],
                              [("den", a, 2)], [("den", a, 3)],
                              [("pv", a, 0), ("pv", a, 1)],
                              [("pv", a, 2), ("pv", a, 3)]]
                projs = []
                if 2 <= c <= n_chunks + 1:
                    projs = [("proj", c - 2, rt) for rt in range(4)]
                big = []
                if c < n_chunks:
                    big = [("qkv", c, 4),  ("qkv", c, 0),       # k0, q0
                           ("qkv", c, 1),  ("qkv", c, 8),       # q1, v0
                           ("qkv", c, 2),  ("qkv", c, 5),       # q2, k1
                           ("qkv", c, 3),  ("qkv", c, 9),       # q3, v1
                           ("qkv", c, 6),  ("qkv", c, 10),      # k2, v2
                           ("qkv", c, 7),  ("qkv", c, 11)]      # k3, v3
                    # pair: (qk0,k0) (qk1,q0) (den0,q1) (qk2,v0) (den1,q2)
                    #       (qk3,k1) (den2,q3) (den3,v1) (pv0,k2) (pv1,v2)
                    #       (pv2,k3) (pv3,v3) then projs
                order = []
                for i in range(max(len(big), len(smalls))):
                    if i < len(smalls):
                        order.extend(smalls[i])
                    if i < len(big):
                        order.append(big[i])
                order += projs
                for kind, cc, idx in order:
                    if kind == "qkv":
                        emit_qkv_group(cc, idx)
                    elif kind == "qk":
                        emit_attn_qk(cc, idx)
                    elif kind == "den":
                        emit_attn_den(cc, idx)
                    elif kind == "pv":
                        emit_attn_pv(cc, idx)
                    else:
                        emit_proj_group(cc, idx)
                if c >= 2:
                    del st[c - 2]
            for out_f32, row in pending_out:
                nc.sync.dma_start(out=out[row:row + 128, :], in_=out_f32[:, :])
            pending_out.clear()
    nc.compile()
    return nc


_NC_CACHE = {}


def _get_nc(n_chunks):
    if n_chunks not in _NC_CACHE:
        _NC_CACHE[n_chunks] = build_nc(n_chunks)
    return _NC_CACHE[n_chunks]


def prep_shared(w_qkv1, b_qkv1, w_qkv2, b_qkv2, bias_table, rel_index, w_proj,
                b_proj, mask):
    """Host-side prep of weights/bias tables shared by all cores."""
    w1 = w_qkv1.astype(BF)
    w2 = np.zeros((CYP, 3 * C), np.float32)
    w2[:CY] = w_qkv2
    # k/v biases ride an all-ones row in the padded region of yT
    w2[CY, C:2 * C] = b_qkv1[C:2 * C] + b_qkv2[C:2 * C]
    w2[CY, 2 * C:] = b_qkv1[2 * C:] + b_qkv2[2 * C:]
    w2 = w2.astype(BF)
    # quad-permuted rows: wp[Q, p, :] = w_proj[32*(4Q + p//32) + p%32, :]
    wp = np.empty((4, 128, C), np.float32)
    for q in range(4):
        for g in range(4):
            wp[q, 32 * g:32 * (g + 1), :] = \
                w_proj[32 * (4 * q + g):32 * (4 * q + g) + 32, :]
    wp = wp.astype(BF)

    bq = b_qkv1[0:C].reshape(FT_Q, 128).T.astype(np.float32).copy()
    bp = np.broadcast_to(b_proj.astype(np.float32), (128, C)).copy()

    rpb = bias_table[rel_index.reshape(-1)].reshape(N, N, H).transpose(2, 0, 1)
    cbt = np.exp(rpb[None] + mask[:, None])            # [w, h, n, m]
    cbt = np.ascontiguousarray(cbt.transpose(0, 1, 3, 2))  # [w, h, m, n]
    cb6 = cbt.reshape(8, 8, 8, 2, N, N)                # [c8, s, hp, hh, m, n]
    cbd = np.ascontiguousarray(cb6.transpose(0, 2, 3, 4, 1, 5)) \
        .reshape(8, 8, 128, 512).astype(BF)

    # den matmul masks, pre-broadcast: out column block 32*hh2..32*hh2+32 of
    # mask (hh2//2) sums key-partition half (hh2%2), so den_rep[m, n] =
    # den(head m//32, n) lands replicated across each 32-partition block
    onesr = np.zeros((128, 2, 128), np.float32)
    for hh2 in range(4):
        hp, hh = hh2 // 2, hh2 % 2
        onesr[64 * hh:64 * hh + 64, hp, 32 * hh2:32 * hh2 + 32] = 1.0
    onesr = onesr.astype(BF)
    return dict(w1=w1, w2=w2, wp=wp, bq=bq, bp=bp, cb=cbd, onesr=onesr)


def prep_core_inputs(x, y, shared, n_cores=N_CORES):
    """Split x, y along batch, transpose to feature-major, bf16."""
    B_, n, _ = x.shape
    rows = (B_ // n_cores) * n
    in_maps = []
    for i in range(n_cores):
        lo = i * (B_ // n_cores)
        hi = lo + B_ // n_cores
        xs = x[lo:hi].reshape(rows, CX)
        ys = y[lo:hi].reshape(rows, CY)
        xtb = np.ascontiguousarray(xs.T).astype(BF)
        ytb = np.zeros((CYP, rows), BF)
        ytb[:CY] = np.ascontiguousarray(ys.T).astype(BF)
        ytb[CY] = 1.0
        in_maps.append(dict(xt=xtb, yt=ytb, **shared))
    return in_maps


def kernel(x, y, mask, w_qkv1, b_qkv1, w_qkv2, b_qkv2, bias_table, rel_index,
           w_proj, b_proj, _n_cores=N_CORES, _trace=False):
    B_, n, _ = x.shape
    n_chunks = (B_ // _n_cores) // WIN_PER_CHUNK
    shared = prep_shared(np.asarray(w_qkv1), np.asarray(b_qkv1),
                         np.asarray(w_qkv2), np.asarray(b_qkv2),
                         np.asarray(bias_table), np.asarray(rel_index),
                         np.asarray(w_proj), np.asarray(b_proj),
                         np.asarray(mask))
    in_maps = prep_core_inputs(np.asarray(x), np.asarray(y), shared, _n_cores)
    nc = _get_nc(n_chunks)
    res = run_bass_kernel_spmd(nc, in_maps, core_ids=list(range(_n_cores)),
                               trace=_trace)
    outs = [res.results[i]["out"].reshape(B_ // _n_cores, n, C)
            for i in range(_n_cores)]
    full = np.concatenate(outs, axis=0)
    kernel.last_results = res
    return full
